# Initial kernel scaffold
#
"""Trainium2 Bass kernel: CNN encoder (conv1d F=8, D=128 -> K=256, valid, + bias + ReLU).

Computation: out[b, l, k] = relu(b_k[k] + sum_{f,d} x[b, l+f, d] * filt[f,d] * W[f*D+d, k])
for l in [0, L-F)  (2040 windows).

Strategy:
  - Data-parallel: 32 batches / 8 cores = 4 batches per core. Params replicated.
  - Host folds filt into W (Wp[f,d,k] = filt[f,d]*W[f*128+d,k]) and transposes x to
    d-major (xT[b, d, l]) so the contraction dim (d=128) lands on SBUF partitions
    with fully-contiguous DMA.
  - On device: for each 512-wide stripe of output positions l and each half of k,
    accumulate 8 matmuls (one per filter tap f) into one PSUM bank:
        psum[k=128p, l=512] += Wp[f,:,kh].T @ xT[:, l0+f : l0+f+512]
    using float32r (full-rate single-pass fp32 matmul; moving dim 512 >= 256).
  - Eviction fuses bias-add + ReLU in one op (bias is per-partition since k is the
    partition dim), alternating ScalarE activation / VectorE tensor_scalar.
  - Output written k-major ([b, k, l]); host transposes back to [b, l, k].
"""

import numpy as np

import concourse.bass as bass
import concourse.tile as tile
import concourse.mybir as mybir
from concourse.bass_utils import run_bass_kernel_spmd

F32 = mybir.dt.float32
F32R = mybir.dt.float32r

N_CORES = 8
B, L, D = 32, 2048, 128
F, K = 8, 256
N_WIN = L - F            # 2040
BP = B // N_CORES        # batches per core
KH = K // 128            # k halves
# output-position stripes per batch: 3x512 + 1x504
SUPERS = [(0, 512), (512, 512), (1024, 512), (1536, N_WIN - 1536)]

# matmul input dtype: F32R = single-pass fp32 (full PE rate at N>=256), F32 = 2-pass
MM_DT = F32R


def _build_program():
    nc = bass.Bass()
    xT_d = nc.declare_dram_parameter("xT", [BP, D, L], F32, isOutput=False)
    wp_d = nc.declare_dram_parameter("wp", [D, F, K], F32, isOutput=False)
    bias_d = nc.declare_dram_parameter("bias", [128, KH], F32, isOutput=False)
    out_d = nc.declare_dram_parameter("outT", [BP, KH, 128, N_WIN], F32, isOutput=True)

    with tile.TileContext(nc) as tc:
        with (
            tc.tile_pool(name="const", bufs=1) as const_pool,
            tc.tile_pool(name="xt", bufs=BP) as xt_pool,
            tc.tile_pool(name="psum", bufs=4, space=bass.MemorySpace.PSUM) as psum_pool,
            tc.tile_pool(name="out", bufs=4) as out_pool,
        ):
            wp_sb = const_pool.tile([D, F, K], F32, tag="wp")
            bias_sb = const_pool.tile([128, KH], F32, tag="bias")
            nc.sync.dma_start(wp_sb[:], wp_d[:])
            nc.sync.dma_start(bias_sb[:], bias_d[:])

            xt_sb = []
            for b in range(BP):
                t = xt_pool.tile([D, L], F32, tag="xt")
                nc.sync.dma_start(t[:], xT_d[b])
                xt_sb.append(t)

            for b in range(BP):
                for l0, ls in SUPERS:
                    for kh in range(KH):
                        ps = psum_pool.tile([128, 512], F32, tag="ps")
                        for f in range(F):
                            nc.tensor.matmul(
                                ps[:, :ls],
                                lhsT=wp_sb[:, f, kh * 128:(kh + 1) * 128].bitcast(MM_DT),
                                rhs=xt_sb[b][:, l0 + f:l0 + f + ls].bitcast(MM_DT),
                                start=(f == 0),
                                stop=(f == F - 1),
                            )
                        ot = out_pool.tile([128, 512], F32, tag="ot")
                        if kh == 0:
                            nc.scalar.activation(
                                ot[:, :ls], ps[:, :ls],
                                mybir.ActivationFunctionType.Relu,
                                bias=bias_sb[:, kh:kh + 1], scale=1.0,
                            )
                        else:
                            nc.vector.tensor_scalar(
                                ot[:, :ls], ps[:, :ls],
                                scalar1=bias_sb[:, kh:kh + 1], scalar2=0.0,
                                op0=mybir.AluOpType.add, op1=mybir.AluOpType.max,
                            )
                        nc.sync.dma_start(out_d[b, kh, :, l0:l0 + ls], ot[:, :ls])
    return nc


def _prep_inputs(user_batch, filt, W_k, b_k):
    user_batch = np.asarray(user_batch, dtype=np.float32)
    filt = np.asarray(filt, dtype=np.float32)
    W_k = np.asarray(W_k, dtype=np.float32)
    b_k = np.asarray(b_k, dtype=np.float32)

    wp = W_k.reshape(F, D, K) * filt[:, :, None]          # [f, d, k]
    wp_host = np.ascontiguousarray(wp.transpose(1, 0, 2))  # [d, f, k]
    bias_host = np.ascontiguousarray(b_k.reshape(KH, 128).T)  # [128, kh]
    xT = np.ascontiguousarray(user_batch.transpose(0, 2, 1))  # [b, d, l]
    return xT, wp_host, bias_host


def _run(user_batch, filt, W_k, b_k, trace=False):
    xT, wp_host, bias_host = _prep_inputs(user_batch, filt, W_k, b_k)
    nc = _build_program()
    in_maps = [
        {"xT": xT[c * BP:(c + 1) * BP], "wp": wp_host, "bias": bias_host}
        for c in range(N_CORES)
    ]
    res = run_bass_kernel_spmd(nc, in_maps, list(range(N_CORES)), trace=trace)
    outT = np.concatenate([r["outT"] for r in res.results], axis=0)  # [B, KH, 128, N_WIN]
    out = outT.reshape(B, K, N_WIN).transpose(0, 2, 1)               # [B, N_WIN, K]
    return np.ascontiguousarray(out), res


def kernel(user_batch, filt, W_k, b_k):
    out, _ = _run(user_batch, filt, W_k, b_k, trace=False)
    return out


# revision 10
# speedup vs baseline: 1.0061x; 1.0061x over previous
"""Trainium2 Bass kernel: CNN encoder (conv1d F=8, D=128 -> K=256, valid, + bias + ReLU).

Computation: out[b, l, k] = relu(b_k[k] + sum_{f,d} x[b, l+f, d] * filt[f,d] * W[f*D+d, k])
for l in [0, L-F)  (2040 windows).

Strategy:
  - Data-parallel: 32 batches / 8 cores = 4 batches per core. Params replicated.
  - Host folds filt into W (Wp[f,d,k] = filt[f,d]*W[f*128+d,k]) and transposes x to
    d-major (xT[b, d, l]) so the contraction dim (d=128) lands on SBUF partitions
    with fully-contiguous DMA.
  - On device: for each 512-wide stripe of output positions l and each half of k,
    accumulate 8 matmuls (one per filter tap f) into one PSUM bank:
        psum[k=128p, l=512] += Wp[f,:,kh].T @ xT[:, l0+f : l0+f+512]
    using float32r (full-rate single-pass fp32 matmul; moving dim 512 >= 256).
  - Eviction fuses bias-add + ReLU in one op (bias is per-partition since k is the
    partition dim), alternating ScalarE activation / VectorE tensor_scalar.
  - Output written k-major ([b, k, l]); host transposes back to [b, l, k].
"""

import numpy as np

import concourse.bacc as bacc
import concourse.bass as bass
import concourse.tile as tile
import concourse.mybir as mybir
from concourse.bass_utils import run_bass_kernel_spmd

F32 = mybir.dt.float32
F32R = mybir.dt.float32r

N_CORES = 8
B, L, D = 32, 2048, 128
F, K = 8, 256
N_WIN = L - F            # 2040
BP = B // N_CORES        # batches per core
KH = K // 128            # k halves
# output-position stripes per batch: 3x512 + 1x504
SUPERS = [(0, 512), (512, 512), (1024, 512), (1536, N_WIN - 1536)]

# matmul input dtype: F32R = single-pass fp32 (full PE rate at N>=256), F32 = 2-pass
MM_DT = F32R


def _build_program(reps=1, loop_n=0):
    """One SPMD program for all 8 cores. reps>1 unrolls the full body (input
    DMAs + compute + output DMAs); rep r writes to output rows [r*BP, (r+1)*BP).
    loop_n>0 additionally wraps the body in a hardware For_i loop (benchmarking
    only: every loop iteration rewrites the same output region)."""
    nc = bacc.Bacc(
        "TRN2",
        target_bir_lowering=False,
        debug=False,
        num_devices=N_CORES,
    )
    xT_d = nc.declare_dram_parameter("xT", [BP, D, L], MM_DT, isOutput=False)
    wp_d = nc.declare_dram_parameter("wp", [D, F, K], MM_DT, isOutput=False)
    bias_d = nc.declare_dram_parameter("bias", [128, KH], F32, isOutput=False)
    out_d = nc.declare_dram_parameter(
        "outT", [reps * BP, KH, 128, N_WIN], F32, isOutput=True)

    def body(nc, tc, pools, r):
        const_pool, xt_pool, psum_pool, out_pool = pools
        wp_sb = const_pool.tile([D, F, K], MM_DT, tag="wp")
        bias_sb = const_pool.tile([128, KH], F32, tag="bias")
        nc.sync.dma_start(wp_sb[:], wp_d[:])
        nc.sync.dma_start(bias_sb[:], bias_d[:])

        xt_sb = []
        for b in range(BP):
            t = xt_pool.tile([D, L], MM_DT, tag="xt")
            nc.sync.dma_start(t[:], xT_d[b])
            xt_sb.append(t)

        for b in range(BP):
            for l0, ls in SUPERS:
                for kh in range(KH):
                    ps = psum_pool.tile([128, 512], F32, tag="ps")
                    for f in range(F):
                        nc.tensor.matmul(
                            ps[:, :ls],
                            lhsT=wp_sb[:, f, kh * 128:(kh + 1) * 128],
                            rhs=xt_sb[b][:, l0 + f:l0 + f + ls],
                            start=(f == 0),
                            stop=(f == F - 1),
                        )
                    ot = out_pool.tile([128, 512], F32, tag="ot")
                    if kh == 0:
                        nc.scalar.activation(
                            ot[:, :ls], ps[:, :ls],
                            mybir.ActivationFunctionType.Relu,
                            bias=bias_sb[:, kh:kh + 1], scale=1.0,
                        )
                    else:
                        nc.vector.tensor_scalar(
                            ot[:, :ls], ps[:, :ls],
                            scalar1=bias_sb[:, kh:kh + 1], scalar2=0.0,
                            op0=mybir.AluOpType.add, op1=mybir.AluOpType.max,
                        )
                    nc.sync.dma_start(
                        out_d[r * BP + b, kh, :, l0:l0 + ls], ot[:, :ls])

    with tile.TileContext(nc) as tc:
        with (
            tc.tile_pool(name="const", bufs=2) as const_pool,
            tc.tile_pool(name="xt", bufs=BP) as xt_pool,
            tc.tile_pool(name="psum", bufs=4, space=bass.MemorySpace.PSUM) as psum_pool,
            tc.tile_pool(name="out", bufs=4) as out_pool,
        ):
            pools = (const_pool, xt_pool, psum_pool, out_pool)
            if loop_n > 0:
                with tc.For_i(0, loop_n, 1,
                              hint_engines=(mybir.EngineType.PE,)):
                    for r in range(reps):
                        body(nc, tc, pools, r)
            else:
                for r in range(reps):
                    body(nc, tc, pools, r)
    nc.compile()
    return nc


def _prep_inputs(user_batch, filt, W_k, b_k):
    user_batch = np.asarray(user_batch, dtype=np.float32)
    filt = np.asarray(filt, dtype=np.float32)
    W_k = np.asarray(W_k, dtype=np.float32)
    b_k = np.asarray(b_k, dtype=np.float32)

    wp = W_k.reshape(F, D, K) * filt[:, :, None]          # [f, d, k]
    wp_host = np.ascontiguousarray(wp.transpose(1, 0, 2))  # [d, f, k]
    bias_host = np.ascontiguousarray(b_k.reshape(KH, 128).T)  # [128, kh]
    xT = np.ascontiguousarray(user_batch.transpose(0, 2, 1))  # [b, d, l]
    return xT, wp_host, bias_host


def _run(user_batch, filt, W_k, b_k, trace=False):
    xT, wp_host, bias_host = _prep_inputs(user_batch, filt, W_k, b_k)
    nc = _build_program()
    in_maps = [
        {"xT": xT[c * BP:(c + 1) * BP], "wp": wp_host, "bias": bias_host}
        for c in range(N_CORES)
    ]
    res = run_bass_kernel_spmd(nc, in_maps, list(range(N_CORES)), trace=trace)
    outT = np.concatenate([r["outT"] for r in res.results], axis=0)  # [B, KH, 128, N_WIN]
    out = outT.reshape(B, K, N_WIN).transpose(0, 2, 1)               # [B, N_WIN, K]
    return np.ascontiguousarray(out), res


def kernel(user_batch, filt, W_k, b_k):
    out, _ = _run(user_batch, filt, W_k, b_k, trace=False)
    return out
